# revision 15
# baseline (speedup 1.0000x reference)
"""CoAttention Trainium2 Bass kernel.

Problem (per batch b):
  v1 = text @ W1                               [T,1]
  v2 = img @ W2                                [I,1]
  v3 = (text * W3^T) @ img^T                   [T,I]
  v  = v1 + v2^T + v3 + bias                   [T,I]
  A_img  = softmax(v, axis=I)
  A_text = softmax(max(v, axis=I), axis=T)
  text_re = A_text^T @ text                    [1,D]
  img_re  = A_img @ img                        [T,D]
  G = concat([text, img_re, text*img_re, text*text_re], -1)   [T,4D]

Sharding: data-parallel over batch B=32 across 8 cores (4 batches/core),
weights replicated.

The kernel is HBM-bandwidth bound: 44 MB/core of traffic at ~330-360 GB/s
per core -> ~125-132 us floor. The schedule is organized so the DMA engines
never idle:
  - stores are split into three streams that become ready at different
    phases of a batch: G0 = text copy (ready at load), G12 = [img_re,
    text*img_re] per 128-row tile (ready per t-half as the attention matrix
    streams out of PE), and G4 = text*text_re (ready only after the full
    batch's max/sum reductions).
  - queues: SP ring carries text loads and ALL stores (G0/G12/G4 — its
    sequencer is otherwise idle, so DMA dispatch holds never stall compute
    issue on ACT), Pool SWDGE carries img cast-loads. Emission order keeps
    the in-order ring from head-blocking on not-yet-ready data.
  - input pools are triple-buffered so loads run ~2 batches ahead.
  - the batch loop is software-pipelined: batch b+1's input-side prep
    (casts, imgT/rt, textT transposes, v2) is emitted during batch b's
    section so it fills PE/ACT slack there; the latency-critical scalar
    chain closing batch b's reductions (Z, 1/Z, text_re scale, broadcast)
    is emitted BEFORE that prep so it is never starved by it.

Device algorithm (transposed [I,T] layout so A_img never needs a
transpose):
  R[d,i]   = W3[d]*imgT[d,i] + W1[d]            (folds v1 into the matmul)
  vT[i,t]  = sum_d R[d,i]*textT[d,t]            (PE, bf16)
  expT     = exp(vT + (v2[i]+bias))             (ACT, bias is per-partition)
  s[t]     = sum_i expT  (PE matmul w/ ones);  img_re = expT^T @ img (PE)
  m'[t]    = max_i expT  (elementwise max over i-slabs + PE transpose +
             free-dim reduce);  A_text = m'/sum(m')  (exp is monotone)
  text_re  = (1/Z) * sum_t m'[t]*text[t,:]      (PE, rank-1 style)
"""

import numpy as np
import ml_dtypes

import concourse.bass as bass
import concourse.mybir as mybir
from concourse import bacc
from concourse.tile import TileContext
from concourse.bass_utils import run_bass_kernel_spmd

B, T, I, D = 32, 1024, 512, 512
N_CORES = 8
BPC = B // N_CORES  # batches per core

F32 = mybir.dt.float32
BF16 = mybir.dt.bfloat16

# build-time tuning knobs (read by _build_bass); _cache key includes them
OPTIONS = {
    "in_bufs": 3,  # text/img input pool slots (prefetch depth)
    "wk_bufs": 2,
    "sm_bufs": 3,  # per-batch working tiles
    "g12_bufs": 8,
    "g4_bufs": 8,
    "g0_q": "sp",
    "g0_defer": 99,  # batches >= this emit their G0 store at the batch tail  # queue for the G[:,0:D]=text store
    "g0_d2d": False,  # G0 as HBM->HBM DMA straight from text_in (no SBUF dep)
    "g12_q": "sp",
    "g12_alt": False,  # alternate G12 tile stores between sp and act rings
    "g12_pair": False,  # store two adjacent t-tiles per DMA (1MB transfers)
    "g12_split": False,
    "loads_early": False,  # emit loads(b+2) before tail_bulk(b) so G4 waits never head-block them
    "g2_from_sbuf": False,  # G2 = gb[0:D]*text (frees ps_ir after G1 alone)  # queue for per-tile G[:,D:3D] stores
    "g4_q": "sp",  # queue for per-tile G[:,3D:4D] stores
    "g4_alt": False,  # alternate G4 group stores between act and sp rings
    "g4_mul": "split",
    "last_split": False,
    "vt_merged": False,
    "mt_early": False,
    "g4_group": 1,  # tiles per G4 store DMA (1|2|4|8)  # engine(s) for text*text_re: dve | pool | split
    "ttcopy": "dve",
    "cast_eng": "dve",  # text f32->bf16 cast: dve | pool  # textT PSUM->SBUF copy engine: act | dve
    "hi_pri_loads": False,
    "img_hwdge": False,  # load img f32 on ACT HWDGE + DVE cast instead of SWDGE cast-DMA
    "ps_vt_bufs": 2,
    "ps_ir_bufs": 2,
    "ps_tr_bufs": 2,
    "ps_sm_bufs": 2,
}

_AF = mybir.ActivationFunctionType
_OP = mybir.AluOpType


def _build_bass(repeats=1):
    nc = bacc.Bacc()

    text_in = nc.dram_tensor("text_in", [BPC, T, D], F32, kind="ExternalInput")
    img_in = nc.dram_tensor("img_in", [BPC, I, D], F32, kind="ExternalInput")
    # host-folded weight constants, packed so each loads with ONE dma
    # consts_f cols: 0:4 w3c | 4:8 w1c | 8:9 bias | 9:137 ident_f | 137:138 ones_f
    consts_f = nc.dram_tensor("consts_f", [128, 138], F32, kind="ExternalInput")
    # consts_b cols: 0:4 w2 | 4:132 ident_b | 132:133 ones_b
    consts_b = nc.dram_tensor("consts_b", [128, 133], BF16, kind="ExternalInput")

    g_out = nc.dram_tensor("g_out", [BPC, T, 4 * D], F32, kind="ExternalOutput")

    NT = T // 128  # 8 t-tiles
    NI = I // 128  # 4 i-tiles
    NDC = D // 128  # 4 d-chunks
    HT = 2  # t halves (pipeline granularity)
    TH = NT // HT  # tiles per half

    def qeng(q):
        return {"sp": nc.sync, "act": nc.scalar, "pool": nc.gpsimd}[q]

    with TileContext(nc) as tc:
        with (
            tc.tile_pool(name="consts", bufs=1) as cpool,
            tc.tile_pool(name="inp", bufs=OPTIONS["in_bufs"]) as ipool,
            tc.tile_pool(name="wk", bufs=OPTIONS["wk_bufs"]) as wpool,
            tc.tile_pool(name="g12", bufs=OPTIONS["g12_bufs"]) as gpool12,
            tc.tile_pool(name="g4", bufs=OPTIONS["g4_bufs"]) as gpool4,
            tc.tile_pool(name="sm", bufs=OPTIONS["sm_bufs"]) as spool,
            tc.tile_pool(
                name="ps_vt", bufs=OPTIONS["ps_vt_bufs"], space="PSUM"
            ) as ps_vtp,
            tc.tile_pool(
                name="ps_ir", bufs=OPTIONS["ps_ir_bufs"], space="PSUM"
            ) as ps_irp,
            tc.tile_pool(
                name="ps_tr", bufs=OPTIONS["ps_tr_bufs"], space="PSUM"
            ) as ps_tr,
            tc.tile_pool(
                name="ps_sm", bufs=OPTIONS["ps_sm_bufs"], space="PSUM"
            ) as ps_sm,
        ):
            c_f = cpool.tile([128, 138], F32)
            c_b = cpool.tile([128, 133], BF16)
            with tc.high_priority():
                nc.sync.dma_start(c_f, consts_f[:, :])
                nc.sync.dma_start(c_b, consts_b[:, :])
            c_w3 = c_f[:, 0:4]
            c_w1 = c_f[:, 4:8]
            c_bias = c_f[:, 8:9]
            c_onesf = c_f[:, 137:138]
            c_w2 = c_b[:, 0:4]
            c_idb = c_b[:, 4:132]
            c_onesb = c_b[:, 132:133]

            def emit_loads(b):
                """Input DMAs + the G0 (=text) store; all ready-at-load."""
                st = {}
                st["text_sb"] = ipool.tile([128, NT, D], F32, tag="text_sb", name="text_sb")
                st["img_bf"] = ipool.tile([128, NI, D], BF16, tag="img_bf", name="img_bf")
                lp = tc.high_priority() if OPTIONS["hi_pri_loads"] else None
                if lp is not None:
                    lp.__enter__()
                nc.sync.dma_start(
                    st["text_sb"], text_in[b].rearrange("(n p) d -> p n d", p=128)
                )
                if OPTIONS["img_hwdge"]:
                    img_f32 = wpool.tile(
                        [128, NI, D], F32, tag="img_f32", name="img_f32"
                    )
                    nc.scalar.dma_start(
                        img_f32, img_in[b].rearrange("(m p) d -> p m d", p=128)
                    )
                    nc.vector.tensor_copy(st["img_bf"], img_f32)
                else:
                    nc.gpsimd.dma_start(
                        st["img_bf"], img_in[b].rearrange("(m p) d -> p m d", p=128)
                    )
                if lp is not None:
                    lp.__exit__(None, None, None)
                if b < OPTIONS["g0_defer"]:
                    if OPTIONS["g0_d2d"]:
                        # HBM->HBM copy straight from the input tensor: no SBUF
                        # dependency, so the ring dispatches it immediately and
                        # it soaks up any DMA idle between store bursts.
                        qeng(OPTIONS["g0_q"]).dma_start(
                            g_out[b, :, 0:D], text_in[b]
                        )
                    else:
                        gv = g_out[b].rearrange("(n p) g -> p n g", p=128)
                        qeng(OPTIONS["g0_q"]).dma_start(gv[:, :, 0:D], st["text_sb"])
                return st

            def emit_prep(b, st):
                """Input-side prep for batch b, emitted one section early so it
                overlaps the previous batch's main phase."""
                text_sb = st["text_sb"]
                img_bf = st["img_bf"]
                text_bf = wpool.tile([128, NT, D], BF16, tag="text_bf")
                (nc.vector if OPTIONS["cast_eng"] == "dve" else nc.gpsimd).tensor_copy(
                    text_bf, text_sb
                )
                st["text_bf"] = text_bf

                rt_bf = wpool.tile([128, NDC, I], BF16, tag="rt_bf")
                imgT_bf = wpool.tile([128, NDC, I], BF16, tag="imgT_bf")
                for c in range(NDC):
                    ps_it = ps_tr.tile([128, I], BF16, tag="tr", name="ps_it")
                    for m in range(NI):
                        nc.tensor.transpose(
                            ps_it[:, m * 128 : (m + 1) * 128],
                            img_bf[:, m, c * 128 : (c + 1) * 128],
                            c_idb,
                        )
                    nc.vector.tensor_scalar(
                        rt_bf[:, c, :],
                        ps_it,
                        c_w3[:, c : c + 1],
                        c_w1[:, c : c + 1],
                        _OP.mult,
                        _OP.add,
                    )
                    nc.scalar.activation(imgT_bf[:, c, :], ps_it, _AF.Copy)
                st["rt_bf"] = rt_bf

                ps_v2 = ps_sm.tile([128, NI], F32, tag="ps", name="ps_v2")
                for m in range(NI):
                    for c in range(NDC):
                        nc.tensor.matmul(
                            ps_v2[:, m : m + 1],
                            imgT_bf[:, c, m * 128 : (m + 1) * 128],
                            c_w2[:, c : c + 1],
                            start=(c == 0),
                            stop=(c == NDC - 1),
                        )
                v2b = spool.tile([128, NI], F32, tag="v2b")
                nc.scalar.activation(v2b, ps_v2, _AF.Identity, bias=c_bias, scale=1.0)
                st["v2b"] = v2b

                textT_bf = wpool.tile([128, NDC, T], BF16, tag="textT_bf")
                for h in range(HT):
                    t0 = h * (T // HT)
                    for c in range(NDC):
                        ps_tt = ps_tr.tile([128, 512], BF16, tag="tr", name="ps_tt")
                        for k in range(TH):
                            n = h * TH + k
                            nc.tensor.transpose(
                                ps_tt[:, k * 128 : (k + 1) * 128],
                                text_bf[:, n, c * 128 : (c + 1) * 128],
                                c_idb,
                            )
                        if OPTIONS["ttcopy"] == "act":
                            nc.scalar.activation(
                                textT_bf[:, c, t0 : t0 + 512], ps_tt, _AF.Copy
                            )
                        else:
                            nc.vector.tensor_copy(textT_bf[:, c, t0 : t0 + 512], ps_tt)
                st["textT_bf"] = textT_bf

            def emit_main(b, st):
                text_sb = st["text_sb"]
                img_bf = st["img_bf"]
                textT_bf = st["textT_bf"]
                rt_bf = st["rt_bf"]
                v2b = st["v2b"]
                expT_bf = wpool.tile([128, NI, T], BF16, tag="expT_bf")
                m8 = spool.tile([128, T], BF16, tag="m8")
                mprime = spool.tile([128, NT], BF16, tag="mprime")
                st["mprime"] = mprime
                ps_trr = ps_sm.tile([1, 512], F32, tag="ps", name="ps_trr")
                st["ps_trr"] = ps_trr
                if OPTIONS["vt_merged"]:
                    # both t-halves per m-slab: consecutive matmuls share the
                    # same stationary rt chunk, halving the Ldweights count
                    for m in range(NI):
                        ps_vt0 = ps_vtp.tile([128, 512], F32, tag="vt", name="ps_vt0")
                        ps_vt1 = ps_vtp.tile([128, 512], F32, tag="vt", name="ps_vt1")
                        for c in range(NDC):
                            nc.tensor.matmul(
                                ps_vt0,
                                rt_bf[:, c, m * 128 : (m + 1) * 128],
                                textT_bf[:, c, 0:512],
                                start=(c == 0),
                                stop=(c == NDC - 1),
                                skip_group_check=True,
                            )
                            nc.tensor.matmul(
                                ps_vt1,
                                rt_bf[:, c, m * 128 : (m + 1) * 128],
                                textT_bf[:, c, 512:1024],
                                start=(c == 0),
                                stop=(c == NDC - 1),
                                skip_group_check=True,
                            )
                        nc.scalar.activation(
                            expT_bf[:, m, 0:512],
                            ps_vt0,
                            _AF.Exp,
                            bias=v2b[:, m : m + 1],
                            scale=1.0,
                        )
                        nc.scalar.activation(
                            expT_bf[:, m, 512:1024],
                            ps_vt1,
                            _AF.Exp,
                            bias=v2b[:, m : m + 1],
                            scale=1.0,
                        )
                for h in range(HT):
                    t0 = h * (T // HT)
                    if not OPTIONS["vt_merged"]:
                        for m in range(NI):
                            ps_vt = ps_vtp.tile([128, 512], F32, tag="vt", name="ps_vt")
                            for c in range(NDC):
                                nc.tensor.matmul(
                                    ps_vt,
                                    rt_bf[:, c, m * 128 : (m + 1) * 128],
                                    textT_bf[:, c, t0 : t0 + 512],
                                    start=(c == 0),
                                    stop=(c == NDC - 1),
                                )
                            nc.scalar.activation(
                                expT_bf[:, m, t0 : t0 + 512],
                                ps_vt,
                                _AF.Exp,
                                bias=v2b[:, m : m + 1],
                                scale=1.0,
                            )
                    mxa = spool.tile([128, T // HT], BF16, tag="mxa")
                    mxb = spool.tile([128, T // HT], BF16, tag="mxb")
                    nc.vector.tensor_max(
                        mxa, expT_bf[:, 0, t0 : t0 + 512], expT_bf[:, 1, t0 : t0 + 512]
                    )
                    nc.vector.tensor_max(
                        mxb, expT_bf[:, 2, t0 : t0 + 512], expT_bf[:, 3, t0 : t0 + 512]
                    )
                    nc.vector.tensor_max(m8[:, t0 : t0 + 512], mxa, mxb)

                    for k in range(TH):
                        n = h * TH + k
                        # ir and s interleaved per m-slab: consecutive
                        # matmuls share the same stationary lhsT, so the
                        # hardware skips the second Ldweights
                        ps_ir = ps_irp.tile([128, D], F32, tag="ir", name="ps_ir")
                        ps_s = ps_sm.tile([128, 1], F32, tag="ps", name="ps_s")
                        for m in range(NI):
                            nc.tensor.matmul(
                                ps_ir,
                                expT_bf[:, m, n * 128 : (n + 1) * 128],
                                img_bf[:, m, :],
                                start=(m == 0),
                                stop=(m == NI - 1),
                                skip_group_check=True,
                            )
                            nc.tensor.matmul(
                                ps_s,
                                expT_bf[:, m, n * 128 : (n + 1) * 128],
                                c_onesb,
                                start=(m == 0),
                                stop=(m == NI - 1),
                                skip_group_check=True,
                            )
                        rs = spool.tile([128, 1], F32, tag="rs")
                        nc.vector.reciprocal(rs, ps_s)

                        if OPTIONS["mt_early"]:
                            ps_mt = ps_sm.tile(
                                [128, 128], BF16, tag="ps", name="ps_mt"
                            )
                            nc.tensor.transpose(
                                ps_mt, m8[:, n * 128 : (n + 1) * 128], c_idb
                            )
                            nc.vector.reduce_max(
                                mprime[:, n : n + 1], ps_mt, axis=mybir.AxisListType.X
                            )
                            nc.tensor.matmul(
                                ps_trr,
                                mprime[:, n : n + 1],
                                st["text_bf"][:, n, :],
                                start=(n == 0),
                                stop=(n == NT - 1),
                                skip_group_check=True,
                            )

                        if OPTIONS["g12_pair"]:
                            if n % 2 == 0:
                                gbp = gpool12.tile([128, 2, 2 * D], F32, tag="g12")
                                st["gbp"] = gbp
                            else:
                                gbp = st["gbp"]
                            gb = gbp[:, n % 2, :]
                        else:
                            gb = gpool12.tile([128, 2 * D], F32, tag="g12")
                        nc.scalar.activation(gb[:, 0:D], ps_ir, _AF.Copy, scale=rs)
                        g2eng = (
                            nc.gpsimd
                            if (b == BPC - 1 and n % 2 == 1 and OPTIONS["last_split"])
                            else nc.vector
                        )
                        if OPTIONS["g2_from_sbuf"]:
                            # text * img_re == text * G1: reading the already
                            # normalized SBUF copy releases the ps_ir bank
                            # after the ACT copy alone, so the next-but-one
                            # tile's matmuls start earlier
                            g2eng.tensor_mul(
                                gb[:, D : 2 * D], gb[:, 0:D], text_sb[:, n, :]
                            )
                        else:
                            g2eng.scalar_tensor_tensor(
                                gb[:, D : 2 * D],
                                ps_ir,
                                rs,
                                text_sb[:, n, :],
                                _OP.mult,
                                _OP.mult,
                            )
                        if OPTIONS["g12_alt"]:
                            g12q = "sp" if n % 2 == 0 else "act"
                        else:
                            g12q = OPTIONS["g12_q"]
                        if OPTIONS["g12_pair"]:
                            if n % 2 == 1:
                                gvp = g_out[b].rearrange("(q p) g -> p q g", p=128)
                                if OPTIONS["g12_alt"]:
                                    g12q = "sp" if (n // 2) % 2 == 0 else "act"
                                qeng(g12q).dma_start(
                                    gvp[:, n - 1 : n + 1, D : 3 * D], st["gbp"]
                                )
                        elif OPTIONS["g12_split"]:
                            qeng(g12q).dma_start(
                                g_out[b, n * 128 : (n + 1) * 128, D : 2 * D],
                                gb[:, 0:D],
                            )
                            qeng(g12q).dma_start(
                                g_out[b, n * 128 : (n + 1) * 128, 2 * D : 3 * D],
                                gb[:, D : 2 * D],
                            )
                        else:
                            qeng(g12q).dma_start(
                                g_out[b, n * 128 : (n + 1) * 128, D : 3 * D], gb
                            )

                        if not OPTIONS["mt_early"]:
                            ps_mt = ps_sm.tile(
                                [128, 128], BF16, tag="ps", name="ps_mt"
                            )
                            nc.tensor.transpose(
                                ps_mt, m8[:, n * 128 : (n + 1) * 128], c_idb
                            )
                            nc.vector.reduce_max(
                                mprime[:, n : n + 1], ps_mt, axis=mybir.AxisListType.X
                            )
                            # accumulate text_re per tile so the batch tail is
                            # only recip/scale/broadcast deep
                            nc.tensor.matmul(
                                ps_trr,
                                mprime[:, n : n + 1],
                                st["text_bf"][:, n, :],
                                start=(n == 0),
                                stop=(n == NT - 1),
                                skip_group_check=True,
                            )

            def emit_tail_fast(b, st):
                """Latency-critical scalar chain closing the batch's
                reductions; tiny engine time, so it outranks the next batch's
                prep in every engine's priority order."""
                mprime = st["mprime"]
                ps_trr = st["ps_trr"]
                mcol = spool.tile([128, 1], F32, tag="mcol")
                nc.vector.reduce_sum(mcol, mprime, axis=mybir.AxisListType.X)
                ps_z = ps_sm.tile([1, 1], F32, tag="ps", name="ps_z")
                nc.tensor.matmul(ps_z, mcol, c_onesf, start=True, stop=True)
                rz = spool.tile([1, 1], F32, tag="rz")
                nc.vector.reciprocal(rz, ps_z)
                trerow = spool.tile([1, 512], F32, tag="trerow")
                nc.scalar.activation(trerow, ps_trr, _AF.Copy, scale=rz)
                bcast = spool.tile([128, 512], F32, tag="bcast")
                nc.gpsimd.partition_broadcast(bcast, trerow)
                st["bcast"] = bcast

            def emit_tail_bulk(b, st):
                text_sb = st["text_sb"]
                bcast = st["bcast"]
                if b >= OPTIONS["g0_defer"]:
                    gv = g_out[b].rearrange("(n p) g -> p n g", p=128)
                    qeng(OPTIONS["g0_q"]).dma_start(gv[:, :, 0:D], text_sb)
                GG = OPTIONS["g4_group"]
                gv4 = g_out[b].rearrange("(q p) g -> p q g", p=128)
                for gi, n0 in enumerate(range(0, NT, GG)):
                    gb4 = gpool4.tile([128, GG, D], F32, tag="g4")
                    for j in range(GG):
                        n = n0 + j
                        mode = OPTIONS["g4_mul"]
                        if b == BPC - 1 and OPTIONS["last_split"]:
                            mode = "split"
                        if mode == "dve":
                            eng = nc.vector
                        elif mode == "pool":
                            eng = nc.gpsimd
                        else:
                            eng = nc.vector if n % 2 == 0 else nc.gpsimd
                        eng.tensor_mul(gb4[:, j, :], text_sb[:, n, :], bcast)
                    if OPTIONS["g4_alt"]:
                        g4q = "act" if gi % 2 == 0 else "sp"
                    else:
                        g4q = OPTIONS["g4_q"]
                    qeng(g4q).dma_start(gv4[:, n0 : n0 + GG, 3 * D : 4 * D], gb4)

            import contextlib

            loop_ctx = (
                tc.For_i(0, repeats, 1) if repeats > 1 else contextlib.nullcontext()
            )
            with loop_ctx:
                # software-pipelined: prep(b+1) is emitted during section b so
                # the next batch's transposes/casts overlap this batch's main
                # attention phase instead of serializing at the boundary.
                states = {0: emit_loads(0)}
                emit_prep(0, states[0])
                if OPTIONS["loads_early"]:
                    # loads(b+2) land on the rings BEFORE tail_bulk(b)'s G4
                    # stores, so the bcast-chain wait can't head-block them
                    if BPC > 1:
                        states[1] = emit_loads(1)
                    for b in range(BPC):
                        emit_main(b, states[b])
                        emit_tail_fast(b, states[b])
                        if b + 2 < BPC:
                            states[b + 2] = emit_loads(b + 2)
                        if b + 1 < BPC:
                            emit_prep(b + 1, states[b + 1])
                        emit_tail_bulk(b, states[b])
                        del states[b]
                else:
                    for b in range(BPC):
                        if b + 1 < BPC:
                            states[b + 1] = emit_loads(b + 1)
                        emit_main(b, states[b])
                        emit_tail_fast(b, states[b])
                        if b + 1 < BPC:
                            emit_prep(b + 1, states[b + 1])
                        emit_tail_bulk(b, states[b])
                        del states[b]

    nc.compile()
    return nc


_cache = {}


def _get_nc(repeats=1):
    key = f"nc{repeats}-" + "-".join(f"{k}={v}" for k, v in sorted(OPTIONS.items()))
    if key not in _cache:
        _cache[key] = _build_bass(repeats)
    return _cache[key]


def _host_consts(W1, W2, W3, bias):
    w3c = W3[:, 0].reshape(4, 128).T.astype(np.float32)
    w1c = W1[:, 0].reshape(4, 128).T.astype(np.float32)
    w2c = W2[:, 0].reshape(4, 128).T.astype(np.float32)
    bias_col = np.full((128, 1), np.float32(bias[0]), dtype=np.float32)
    ident = np.eye(128, dtype=np.float32)
    ones = np.ones((128, 1), dtype=np.float32)
    consts_f = np.ascontiguousarray(
        np.concatenate([w3c, w1c, bias_col, ident, ones], axis=1, dtype=np.float32)
    )
    consts_b = np.ascontiguousarray(
        np.concatenate([w2c, ident, ones], axis=1).astype(ml_dtypes.bfloat16)
    )
    return dict(consts_f=consts_f, consts_b=consts_b)


def _run(inputs, trace=False, trace_kwargs=None):
    text = np.ascontiguousarray(np.asarray(inputs["text"], dtype=np.float32))
    img = np.ascontiguousarray(np.asarray(inputs["img"], dtype=np.float32))
    consts = _host_consts(
        np.asarray(inputs["W1"], dtype=np.float32),
        np.asarray(inputs["W2"], dtype=np.float32),
        np.asarray(inputs["W3"], dtype=np.float32),
        np.asarray(inputs["bias"], dtype=np.float32),
    )
    nc = _get_nc()
    in_maps = []
    for core in range(N_CORES):
        sl = slice(core * BPC, (core + 1) * BPC)
        in_maps.append(
            dict(
                text_in=np.ascontiguousarray(text[sl]),
                img_in=np.ascontiguousarray(img[sl]),
                **consts,
            )
        )
    kwargs = {}
    if trace:
        kwargs["trace"] = True
        if trace_kwargs:
            kwargs["trace_kwargs"] = trace_kwargs
    # The axon terminal is occasionally left in an "accelerator device
    # unrecoverable" state by a previous process; a backend reset + retry
    # reconnects to a healthy worker.
    last_exc = None
    for attempt in range(3):
        try:
            res = run_bass_kernel_spmd(
                nc, in_maps, core_ids=list(range(N_CORES)), **kwargs
            )
            break
        except Exception as e:  # noqa: BLE001
            last_exc = e
            if "UNRECOVERABLE" not in str(e) and "UNAVAILABLE" not in str(e):
                raise
            try:
                import jax
                import time as _time

                jax.clear_caches()
                jax._src.api.clear_backends()
                _time.sleep(5.0 * (attempt + 1))
            except Exception:
                pass
    else:
        raise last_exc
    out = np.concatenate([r["g_out"] for r in res.results], axis=0)
    return out, res


def kernel(**inputs) -> np.ndarray:
    out, _ = _run(inputs, trace=False)
    return out



# revision 25
# speedup vs baseline: 1.0615x; 1.0615x over previous
"""CoAttention Trainium2 Bass kernel.

Problem (per batch b):
  v1 = text @ W1                               [T,1]
  v2 = img @ W2                                [I,1]
  v3 = (text * W3^T) @ img^T                   [T,I]
  v  = v1 + v2^T + v3 + bias                   [T,I]
  A_img  = softmax(v, axis=I)
  A_text = softmax(max(v, axis=I), axis=T)
  text_re = A_text^T @ text                    [1,D]
  img_re  = A_img @ img                        [T,D]
  G = concat([text, img_re, text*img_re, text*text_re], -1)   [T,4D]

Sharding: data-parallel over batch B=32 across 8 cores (4 batches/core),
weights replicated.

The kernel is HBM-bandwidth bound: 44 MB/core of traffic at ~330-360 GB/s
per core -> ~125-132 us floor. The schedule is organized so the DMA engines
never idle:
  - stores are split into three streams that become ready at different
    phases of a batch: G0 = text copy (ready at load), G12 = [img_re,
    text*img_re] per 128-row tile (ready per t-half as the attention matrix
    streams out of PE), and G4 = text*text_re (ready only after the full
    batch's max/sum reductions).
  - queues: SP ring carries text loads + G0/G12 stores (its sequencer is
    otherwise idle, so DMA dispatch holds never stall compute issue on
    ACT); Pool SWDGE carries img cast-loads and the G4 tail stores
    (grouped 2 tiles/DMA) so the batch-end bcast wait never head-blocks
    the SP ring. A pure-DMA probe (dma_probe.py) showed this split
    streams the full 44MB at the HBM floor (~122us) vs ~140us all-SP.
    Data-dependent stores must NEVER go on the ACT HWDGE ring: the ACT
    sequencer stalls on their readiness waits and the exp activations
    starve (+36us measured). DRAM->DRAM G0 is also a trap (per-row 2KB
    descriptors, +86us).
  - input pools are triple-buffered so loads run ~2 batches ahead;
    loads(b+2) are emitted before tail_bulk(b) so G4's bcast-chain wait
    cannot delay their ring dispatch.
  - the batch loop is software-pipelined: batch b+1's input-side prep
    (casts, imgT/rt, textT transposes, v2) is emitted during batch b's
    section so it fills PE/ACT slack there; the latency-critical scalar
    chain closing batch b's reductions (Z, 1/Z, text_re scale, broadcast)
    is emitted BEFORE that prep so it is never starved by it.

Device algorithm (transposed [I,T] layout so A_img never needs a
transpose):
  R[d,i]   = W3[d]*imgT[d,i] + W1[d]            (folds v1 into the matmul)
  vT[i,t]  = sum_d R[d,i]*textT[d,t]            (PE, bf16)
  expT     = exp(vT + (v2[i]+bias))             (ACT, bias is per-partition)
  s[t]     = sum_i expT  (PE matmul w/ ones);  img_re = expT^T @ img (PE)
  m'[t]    = max_i expT  (elementwise max over i-slabs + PE transpose +
             free-dim reduce);  A_text = m'/sum(m')  (exp is monotone)
  text_re  = (1/Z) * sum_t m'[t]*text[t,:]      (PE, rank-1 style)
"""

import numpy as np
import ml_dtypes

import concourse.bass as bass
import concourse.mybir as mybir
from concourse import bacc
from concourse.tile import TileContext
from concourse.bass_utils import run_bass_kernel_spmd

B, T, I, D = 32, 1024, 512, 512
N_CORES = 8
BPC = B // N_CORES  # batches per core

F32 = mybir.dt.float32
BF16 = mybir.dt.bfloat16

# build-time tuning knobs (read by _build_bass); _cache key includes them
OPTIONS = {
    "in_bufs": 3,  # text/img input pool slots (prefetch depth)
    "wk_bufs": 2,
    "sm_bufs": 3,  # per-batch working tiles
    "g12_bufs": 8,
    "g4_bufs": 4,
    "g0_q": "sp",
    "text_q": "sp",  # queue for the text load DMA
    "g0_defer": 99,  # batches >= this emit their G0 store at the batch tail  # queue for the G[:,0:D]=text store
    "g0_d2d": False,  # G0 as HBM->HBM DMA straight from text_in (no SBUF dep)
    "g12_q": "sp",
    "g12_alt": False,  # alternate G12 tile stores between sp and act rings
    "g12_pair": False,  # store two adjacent t-tiles per DMA (1MB transfers)
    "g12_split": False,
    "loads_early": True,  # emit loads(b+2) before tail_bulk(b) so G4 waits never head-block them
    "interleave": False,  # inject L(b+1)/G0(b+1) between G12 stores of main(b)
    "il_text_at": 1,  # G12 tile index after which the text load is injected
    "il_g0_at": 4,  # G12 tile index after which the G0 store is injected
    "g2_from_sbuf": False,  # G2 = gb[0:D]*text (frees ps_ir after G1 alone)  # queue for per-tile G[:,D:3D] stores
    "g4_q": "pool",  # queue for per-tile G[:,3D:4D] stores
    "g4_alt": False,  # alternate G4 group stores between act and sp rings
    "g4_mul": "split",
    "last_split": False,
    "vt_merged": False,
    "mt_early": False,
    "g4_group": 2,  # tiles per G4 store DMA (1|2|4|8)  # engine(s) for text*text_re: dve | pool | split
    "ttcopy": "dve",
    "cast_eng": "dve",  # text f32->bf16 cast: dve | pool  # textT PSUM->SBUF copy engine: act | dve
    "hi_pri_loads": False,
    "img_hwdge": False,  # load img f32 on ACT HWDGE + DVE cast instead of SWDGE cast-DMA
    "ps_vt_bufs": 2,
    "ps_ir_bufs": 2,
    "ps_tr_bufs": 2,
    "ps_sm_bufs": 2,
}

_AF = mybir.ActivationFunctionType
_OP = mybir.AluOpType


def _build_bass(repeats=1):
    nc = bacc.Bacc()

    text_in = nc.dram_tensor("text_in", [BPC, T, D], F32, kind="ExternalInput")
    img_in = nc.dram_tensor("img_in", [BPC, I, D], F32, kind="ExternalInput")
    # host-folded weight constants, packed so each loads with ONE dma
    # consts_f cols: 0:4 w3c | 4:8 w1c | 8:9 bias | 9:137 ident_f | 137:138 ones_f
    consts_f = nc.dram_tensor("consts_f", [128, 138], F32, kind="ExternalInput")
    # consts_b cols: 0:4 w2 | 4:132 ident_b | 132:133 ones_b
    consts_b = nc.dram_tensor("consts_b", [128, 133], BF16, kind="ExternalInput")

    g_out = nc.dram_tensor("g_out", [BPC, T, 4 * D], F32, kind="ExternalOutput")

    NT = T // 128  # 8 t-tiles
    NI = I // 128  # 4 i-tiles
    NDC = D // 128  # 4 d-chunks
    HT = 2  # t halves (pipeline granularity)
    TH = NT // HT  # tiles per half

    def qeng(q):
        return {"sp": nc.sync, "act": nc.scalar, "pool": nc.gpsimd}[q]

    with TileContext(nc) as tc:
        with (
            tc.tile_pool(name="consts", bufs=1) as cpool,
            tc.tile_pool(name="inp", bufs=OPTIONS["in_bufs"]) as ipool,
            tc.tile_pool(name="wk", bufs=OPTIONS["wk_bufs"]) as wpool,
            tc.tile_pool(name="g12", bufs=OPTIONS["g12_bufs"]) as gpool12,
            tc.tile_pool(name="g4", bufs=OPTIONS["g4_bufs"]) as gpool4,
            tc.tile_pool(name="sm", bufs=OPTIONS["sm_bufs"]) as spool,
            tc.tile_pool(
                name="ps_vt", bufs=OPTIONS["ps_vt_bufs"], space="PSUM"
            ) as ps_vtp,
            tc.tile_pool(
                name="ps_ir", bufs=OPTIONS["ps_ir_bufs"], space="PSUM"
            ) as ps_irp,
            tc.tile_pool(
                name="ps_tr", bufs=OPTIONS["ps_tr_bufs"], space="PSUM"
            ) as ps_tr,
            tc.tile_pool(
                name="ps_sm", bufs=OPTIONS["ps_sm_bufs"], space="PSUM"
            ) as ps_sm,
        ):
            c_f = cpool.tile([128, 138], F32)
            c_b = cpool.tile([128, 133], BF16)
            with tc.high_priority():
                nc.sync.dma_start(c_f, consts_f[:, :])
                nc.sync.dma_start(c_b, consts_b[:, :])
            c_w3 = c_f[:, 0:4]
            c_w1 = c_f[:, 4:8]
            c_bias = c_f[:, 8:9]
            c_onesf = c_f[:, 137:138]
            c_w2 = c_b[:, 0:4]
            c_idb = c_b[:, 4:132]
            c_onesb = c_b[:, 132:133]

            def emit_text_load(b, st):
                qeng(OPTIONS["text_q"]).dma_start(
                    st["text_sb"], text_in[b].rearrange("(n p) d -> p n d", p=128)
                )

            def emit_g0(b, st):
                if OPTIONS["g0_d2d"]:
                    qeng(OPTIONS["g0_q"]).dma_start(g_out[b, :, 0:D], text_in[b])
                else:
                    gv = g_out[b].rearrange("(n p) g -> p n g", p=128)
                    qeng(OPTIONS["g0_q"]).dma_start(gv[:, :, 0:D], st["text_sb"])

            def emit_loads(b, defer_text=False):
                """Input DMAs + the G0 (=text) store; all ready-at-load."""
                st = {}
                st["text_sb"] = ipool.tile([128, NT, D], F32, tag="text_sb", name="text_sb")
                st["img_bf"] = ipool.tile([128, NI, D], BF16, tag="img_bf", name="img_bf")
                lp = tc.high_priority() if OPTIONS["hi_pri_loads"] else None
                if lp is not None:
                    lp.__enter__()
                if not defer_text:
                    emit_text_load(b, st)
                if OPTIONS["img_hwdge"]:
                    img_f32 = wpool.tile(
                        [128, NI, D], F32, tag="img_f32", name="img_f32"
                    )
                    nc.scalar.dma_start(
                        img_f32, img_in[b].rearrange("(m p) d -> p m d", p=128)
                    )
                    nc.vector.tensor_copy(st["img_bf"], img_f32)
                else:
                    nc.gpsimd.dma_start(
                        st["img_bf"], img_in[b].rearrange("(m p) d -> p m d", p=128)
                    )
                if lp is not None:
                    lp.__exit__(None, None, None)
                if b < OPTIONS["g0_defer"] and not defer_text:
                    emit_g0(b, st)
                return st

            def emit_prep(b, st):
                """Input-side prep for batch b, emitted one section early so it
                overlaps the previous batch's main phase."""
                text_sb = st["text_sb"]
                img_bf = st["img_bf"]
                text_bf = wpool.tile([128, NT, D], BF16, tag="text_bf")
                (nc.vector if OPTIONS["cast_eng"] == "dve" else nc.gpsimd).tensor_copy(
                    text_bf, text_sb
                )
                st["text_bf"] = text_bf

                rt_bf = wpool.tile([128, NDC, I], BF16, tag="rt_bf")
                imgT_bf = wpool.tile([128, NDC, I], BF16, tag="imgT_bf")
                for c in range(NDC):
                    ps_it = ps_tr.tile([128, I], BF16, tag="tr", name="ps_it")
                    for m in range(NI):
                        nc.tensor.transpose(
                            ps_it[:, m * 128 : (m + 1) * 128],
                            img_bf[:, m, c * 128 : (c + 1) * 128],
                            c_idb,
                        )
                    nc.vector.tensor_scalar(
                        rt_bf[:, c, :],
                        ps_it,
                        c_w3[:, c : c + 1],
                        c_w1[:, c : c + 1],
                        _OP.mult,
                        _OP.add,
                    )
                    nc.scalar.activation(imgT_bf[:, c, :], ps_it, _AF.Copy)
                st["rt_bf"] = rt_bf

                ps_v2 = ps_sm.tile([128, NI], F32, tag="ps", name="ps_v2")
                for m in range(NI):
                    for c in range(NDC):
                        nc.tensor.matmul(
                            ps_v2[:, m : m + 1],
                            imgT_bf[:, c, m * 128 : (m + 1) * 128],
                            c_w2[:, c : c + 1],
                            start=(c == 0),
                            stop=(c == NDC - 1),
                        )
                v2b = spool.tile([128, NI], F32, tag="v2b")
                nc.scalar.activation(v2b, ps_v2, _AF.Identity, bias=c_bias, scale=1.0)
                st["v2b"] = v2b

                textT_bf = wpool.tile([128, NDC, T], BF16, tag="textT_bf")
                for h in range(HT):
                    t0 = h * (T // HT)
                    for c in range(NDC):
                        ps_tt = ps_tr.tile([128, 512], BF16, tag="tr", name="ps_tt")
                        for k in range(TH):
                            n = h * TH + k
                            nc.tensor.transpose(
                                ps_tt[:, k * 128 : (k + 1) * 128],
                                text_bf[:, n, c * 128 : (c + 1) * 128],
                                c_idb,
                            )
                        if OPTIONS["ttcopy"] == "act":
                            nc.scalar.activation(
                                textT_bf[:, c, t0 : t0 + 512], ps_tt, _AF.Copy
                            )
                        else:
                            nc.vector.tensor_copy(textT_bf[:, c, t0 : t0 + 512], ps_tt)
                st["textT_bf"] = textT_bf

            def emit_main(b, st, fillers=None):
                text_sb = st["text_sb"]
                img_bf = st["img_bf"]
                textT_bf = st["textT_bf"]
                rt_bf = st["rt_bf"]
                v2b = st["v2b"]
                expT_bf = wpool.tile([128, NI, T], BF16, tag="expT_bf")
                m8 = spool.tile([128, T], BF16, tag="m8")
                mprime = spool.tile([128, NT], BF16, tag="mprime")
                st["mprime"] = mprime
                ps_trr = ps_sm.tile([1, 512], F32, tag="ps", name="ps_trr")
                st["ps_trr"] = ps_trr
                if OPTIONS["vt_merged"]:
                    # both t-halves per m-slab: consecutive matmuls share the
                    # same stationary rt chunk, halving the Ldweights count
                    for m in range(NI):
                        ps_vt0 = ps_vtp.tile([128, 512], F32, tag="vt", name="ps_vt0")
                        ps_vt1 = ps_vtp.tile([128, 512], F32, tag="vt", name="ps_vt1")
                        for c in range(NDC):
                            nc.tensor.matmul(
                                ps_vt0,
                                rt_bf[:, c, m * 128 : (m + 1) * 128],
                                textT_bf[:, c, 0:512],
                                start=(c == 0),
                                stop=(c == NDC - 1),
                                skip_group_check=True,
                            )
                            nc.tensor.matmul(
                                ps_vt1,
                                rt_bf[:, c, m * 128 : (m + 1) * 128],
                                textT_bf[:, c, 512:1024],
                                start=(c == 0),
                                stop=(c == NDC - 1),
                                skip_group_check=True,
                            )
                        nc.scalar.activation(
                            expT_bf[:, m, 0:512],
                            ps_vt0,
                            _AF.Exp,
                            bias=v2b[:, m : m + 1],
                            scale=1.0,
                        )
                        nc.scalar.activation(
                            expT_bf[:, m, 512:1024],
                            ps_vt1,
                            _AF.Exp,
                            bias=v2b[:, m : m + 1],
                            scale=1.0,
                        )
                for h in range(HT):
                    t0 = h * (T // HT)
                    if not OPTIONS["vt_merged"]:
                        for m in range(NI):
                            ps_vt = ps_vtp.tile([128, 512], F32, tag="vt", name="ps_vt")
                            for c in range(NDC):
                                nc.tensor.matmul(
                                    ps_vt,
                                    rt_bf[:, c, m * 128 : (m + 1) * 128],
                                    textT_bf[:, c, t0 : t0 + 512],
                                    start=(c == 0),
                                    stop=(c == NDC - 1),
                                )
                            nc.scalar.activation(
                                expT_bf[:, m, t0 : t0 + 512],
                                ps_vt,
                                _AF.Exp,
                                bias=v2b[:, m : m + 1],
                                scale=1.0,
                            )
                    mxa = spool.tile([128, T // HT], BF16, tag="mxa")
                    mxb = spool.tile([128, T // HT], BF16, tag="mxb")
                    nc.vector.tensor_max(
                        mxa, expT_bf[:, 0, t0 : t0 + 512], expT_bf[:, 1, t0 : t0 + 512]
                    )
                    nc.vector.tensor_max(
                        mxb, expT_bf[:, 2, t0 : t0 + 512], expT_bf[:, 3, t0 : t0 + 512]
                    )
                    nc.vector.tensor_max(m8[:, t0 : t0 + 512], mxa, mxb)

                    for k in range(TH):
                        n = h * TH + k
                        # ir and s interleaved per m-slab: consecutive
                        # matmuls share the same stationary lhsT, so the
                        # hardware skips the second Ldweights
                        ps_ir = ps_irp.tile([128, D], F32, tag="ir", name="ps_ir")
                        ps_s = ps_sm.tile([128, 1], F32, tag="ps", name="ps_s")
                        for m in range(NI):
                            nc.tensor.matmul(
                                ps_ir,
                                expT_bf[:, m, n * 128 : (n + 1) * 128],
                                img_bf[:, m, :],
                                start=(m == 0),
                                stop=(m == NI - 1),
                                skip_group_check=True,
                            )
                            nc.tensor.matmul(
                                ps_s,
                                expT_bf[:, m, n * 128 : (n + 1) * 128],
                                c_onesb,
                                start=(m == 0),
                                stop=(m == NI - 1),
                                skip_group_check=True,
                            )
                        rs = spool.tile([128, 1], F32, tag="rs")
                        nc.vector.reciprocal(rs, ps_s)

                        if OPTIONS["mt_early"]:
                            ps_mt = ps_sm.tile(
                                [128, 128], BF16, tag="ps", name="ps_mt"
                            )
                            nc.tensor.transpose(
                                ps_mt, m8[:, n * 128 : (n + 1) * 128], c_idb
                            )
                            nc.vector.reduce_max(
                                mprime[:, n : n + 1], ps_mt, axis=mybir.AxisListType.X
                            )
                            nc.tensor.matmul(
                                ps_trr,
                                mprime[:, n : n + 1],
                                st["text_bf"][:, n, :],
                                start=(n == 0),
                                stop=(n == NT - 1),
                                skip_group_check=True,
                            )

                        if OPTIONS["g12_pair"]:
                            if n % 2 == 0:
                                gbp = gpool12.tile([128, 2, 2 * D], F32, tag="g12")
                                st["gbp"] = gbp
                            else:
                                gbp = st["gbp"]
                            gb = gbp[:, n % 2, :]
                        else:
                            gb = gpool12.tile([128, 2 * D], F32, tag="g12")
                        nc.scalar.activation(gb[:, 0:D], ps_ir, _AF.Copy, scale=rs)
                        g2eng = (
                            nc.gpsimd
                            if (b == BPC - 1 and n % 2 == 1 and OPTIONS["last_split"])
                            else nc.vector
                        )
                        if OPTIONS["g2_from_sbuf"]:
                            # text * img_re == text * G1: reading the already
                            # normalized SBUF copy releases the ps_ir bank
                            # after the ACT copy alone, so the next-but-one
                            # tile's matmuls start earlier
                            g2eng.tensor_mul(
                                gb[:, D : 2 * D], gb[:, 0:D], text_sb[:, n, :]
                            )
                        else:
                            g2eng.scalar_tensor_tensor(
                                gb[:, D : 2 * D],
                                ps_ir,
                                rs,
                                text_sb[:, n, :],
                                _OP.mult,
                                _OP.mult,
                            )
                        if OPTIONS["g12_alt"]:
                            g12q = "sp" if n % 2 == 0 else "act"
                        else:
                            g12q = OPTIONS["g12_q"]
                        if OPTIONS["g12_pair"]:
                            if n % 2 == 1:
                                gvp = g_out[b].rearrange("(q p) g -> p q g", p=128)
                                if OPTIONS["g12_alt"]:
                                    g12q = "sp" if (n // 2) % 2 == 0 else "act"
                                qeng(g12q).dma_start(
                                    gvp[:, n - 1 : n + 1, D : 3 * D], st["gbp"]
                                )
                        elif OPTIONS["g12_split"]:
                            qeng(g12q).dma_start(
                                g_out[b, n * 128 : (n + 1) * 128, D : 2 * D],
                                gb[:, 0:D],
                            )
                            qeng(g12q).dma_start(
                                g_out[b, n * 128 : (n + 1) * 128, 2 * D : 3 * D],
                                gb[:, D : 2 * D],
                            )
                        else:
                            qeng(g12q).dma_start(
                                g_out[b, n * 128 : (n + 1) * 128, D : 3 * D], gb
                            )

                        if fillers and n in fillers:
                            # inject ready-at-dispatch ring work (next batch's
                            # text load / G0 store) between G12 stores so the
                            # in-order SP ring has buffered SDMA work during
                            # the per-tile readiness waits
                            fillers[n]()

                        if not OPTIONS["mt_early"]:
                            ps_mt = ps_sm.tile(
                                [128, 128], BF16, tag="ps", name="ps_mt"
                            )
                            nc.tensor.transpose(
                                ps_mt, m8[:, n * 128 : (n + 1) * 128], c_idb
                            )
                            nc.vector.reduce_max(
                                mprime[:, n : n + 1], ps_mt, axis=mybir.AxisListType.X
                            )
                            # accumulate text_re per tile so the batch tail is
                            # only recip/scale/broadcast deep
                            nc.tensor.matmul(
                                ps_trr,
                                mprime[:, n : n + 1],
                                st["text_bf"][:, n, :],
                                start=(n == 0),
                                stop=(n == NT - 1),
                                skip_group_check=True,
                            )

            def emit_tail_fast(b, st):
                """Latency-critical scalar chain closing the batch's
                reductions; tiny engine time, so it outranks the next batch's
                prep in every engine's priority order."""
                mprime = st["mprime"]
                ps_trr = st["ps_trr"]
                mcol = spool.tile([128, 1], F32, tag="mcol")
                nc.vector.reduce_sum(mcol, mprime, axis=mybir.AxisListType.X)
                ps_z = ps_sm.tile([1, 1], F32, tag="ps", name="ps_z")
                nc.tensor.matmul(ps_z, mcol, c_onesf, start=True, stop=True)
                rz = spool.tile([1, 1], F32, tag="rz")
                nc.vector.reciprocal(rz, ps_z)
                trerow = spool.tile([1, 512], F32, tag="trerow")
                nc.scalar.activation(trerow, ps_trr, _AF.Copy, scale=rz)
                bcast = spool.tile([128, 512], F32, tag="bcast")
                nc.gpsimd.partition_broadcast(bcast, trerow)
                st["bcast"] = bcast

            def emit_tail_bulk(b, st):
                text_sb = st["text_sb"]
                bcast = st["bcast"]
                if b >= OPTIONS["g0_defer"]:
                    gv = g_out[b].rearrange("(n p) g -> p n g", p=128)
                    qeng(OPTIONS["g0_q"]).dma_start(gv[:, :, 0:D], text_sb)
                GG = OPTIONS["g4_group"]
                gv4 = g_out[b].rearrange("(q p) g -> p q g", p=128)
                for gi, n0 in enumerate(range(0, NT, GG)):
                    gb4 = gpool4.tile([128, GG, D], F32, tag="g4")
                    for j in range(GG):
                        n = n0 + j
                        mode = OPTIONS["g4_mul"]
                        if b == BPC - 1 and OPTIONS["last_split"]:
                            mode = "split"
                        if mode == "dve":
                            eng = nc.vector
                        elif mode == "pool":
                            eng = nc.gpsimd
                        else:
                            eng = nc.vector if n % 2 == 0 else nc.gpsimd
                        eng.tensor_mul(gb4[:, j, :], text_sb[:, n, :], bcast)
                    if OPTIONS["g4_alt"]:
                        g4q = "act" if gi % 2 == 0 else "sp"
                    else:
                        g4q = OPTIONS["g4_q"]
                    qeng(g4q).dma_start(gv4[:, n0 : n0 + GG, 3 * D : 4 * D], gb4)

            import contextlib

            loop_ctx = (
                tc.For_i(0, repeats, 1) if repeats > 1 else contextlib.nullcontext()
            )
            with loop_ctx:
                # software-pipelined: prep(b+1) is emitted during section b so
                # the next batch's transposes/casts overlap this batch's main
                # attention phase instead of serializing at the boundary.
                states = {0: emit_loads(0)}
                emit_prep(0, states[0])
                if OPTIONS["interleave"]:
                    # distance-1 prefetch with the text load / G0 store ring
                    # entries injected BETWEEN G12 stores of the current
                    # batch's main phase (in-order ring gap filling)
                    for b in range(BPC):
                        fillers = None
                        if b + 1 < BPC:
                            nst = states[b + 1] = emit_loads(
                                b + 1, defer_text=True
                            )
                            fillers = {
                                OPTIONS["il_text_at"]: (
                                    lambda b=b + 1, s=nst: emit_text_load(b, s)
                                ),
                                OPTIONS["il_g0_at"]: (
                                    lambda b=b + 1, s=nst: emit_g0(b, s)
                                ),
                            }
                        emit_main(b, states[b], fillers)
                        emit_tail_fast(b, states[b])
                        if b + 1 < BPC:
                            emit_prep(b + 1, states[b + 1])
                        emit_tail_bulk(b, states[b])
                        del states[b]
                elif OPTIONS["loads_early"]:
                    # loads(b+2) land on the rings BEFORE tail_bulk(b)'s G4
                    # stores, so the bcast-chain wait can't head-block them
                    if BPC > 1:
                        states[1] = emit_loads(1)
                    for b in range(BPC):
                        emit_main(b, states[b])
                        emit_tail_fast(b, states[b])
                        if b + 2 < BPC:
                            states[b + 2] = emit_loads(b + 2)
                        if b + 1 < BPC:
                            emit_prep(b + 1, states[b + 1])
                        emit_tail_bulk(b, states[b])
                        del states[b]
                else:
                    for b in range(BPC):
                        if b + 1 < BPC:
                            states[b + 1] = emit_loads(b + 1)
                        emit_main(b, states[b])
                        emit_tail_fast(b, states[b])
                        if b + 1 < BPC:
                            emit_prep(b + 1, states[b + 1])
                        emit_tail_bulk(b, states[b])
                        del states[b]

    nc.compile()
    return nc


_cache = {}


def _get_nc(repeats=1):
    key = f"nc{repeats}-" + "-".join(f"{k}={v}" for k, v in sorted(OPTIONS.items()))
    if key not in _cache:
        _cache[key] = _build_bass(repeats)
    return _cache[key]


def _host_consts(W1, W2, W3, bias):
    w3c = W3[:, 0].reshape(4, 128).T.astype(np.float32)
    w1c = W1[:, 0].reshape(4, 128).T.astype(np.float32)
    w2c = W2[:, 0].reshape(4, 128).T.astype(np.float32)
    bias_col = np.full((128, 1), np.float32(bias[0]), dtype=np.float32)
    ident = np.eye(128, dtype=np.float32)
    ones = np.ones((128, 1), dtype=np.float32)
    consts_f = np.ascontiguousarray(
        np.concatenate([w3c, w1c, bias_col, ident, ones], axis=1, dtype=np.float32)
    )
    consts_b = np.ascontiguousarray(
        np.concatenate([w2c, ident, ones], axis=1).astype(ml_dtypes.bfloat16)
    )
    return dict(consts_f=consts_f, consts_b=consts_b)


def _run(inputs, trace=False, trace_kwargs=None):
    text = np.ascontiguousarray(np.asarray(inputs["text"], dtype=np.float32))
    img = np.ascontiguousarray(np.asarray(inputs["img"], dtype=np.float32))
    consts = _host_consts(
        np.asarray(inputs["W1"], dtype=np.float32),
        np.asarray(inputs["W2"], dtype=np.float32),
        np.asarray(inputs["W3"], dtype=np.float32),
        np.asarray(inputs["bias"], dtype=np.float32),
    )
    nc = _get_nc()
    in_maps = []
    for core in range(N_CORES):
        sl = slice(core * BPC, (core + 1) * BPC)
        in_maps.append(
            dict(
                text_in=np.ascontiguousarray(text[sl]),
                img_in=np.ascontiguousarray(img[sl]),
                **consts,
            )
        )
    kwargs = {}
    if trace:
        kwargs["trace"] = True
        if trace_kwargs:
            kwargs["trace_kwargs"] = trace_kwargs
    # The axon terminal is occasionally left in an "accelerator device
    # unrecoverable" state by a previous process; a backend reset + retry
    # reconnects to a healthy worker.
    last_exc = None
    for attempt in range(3):
        try:
            res = run_bass_kernel_spmd(
                nc, in_maps, core_ids=list(range(N_CORES)), **kwargs
            )
            break
        except Exception as e:  # noqa: BLE001
            last_exc = e
            if "UNRECOVERABLE" not in str(e) and "UNAVAILABLE" not in str(e):
                raise
            try:
                import jax
                import time as _time

                jax.clear_caches()
                jax._src.api.clear_backends()
                _time.sleep(5.0 * (attempt + 1))
            except Exception:
                pass
    else:
        raise last_exc
    out = np.concatenate([r["g_out"] for r in res.results], axis=0)
    return out, res


def kernel(**inputs) -> np.ndarray:
    out, _ = _run(inputs, trace=False)
    return out



# revision 29
# speedup vs baseline: 1.1022x; 1.0383x over previous
"""CoAttention Trainium2 Bass kernel.

Problem (per batch b):
  v1 = text @ W1                               [T,1]
  v2 = img @ W2                                [I,1]
  v3 = (text * W3^T) @ img^T                   [T,I]
  v  = v1 + v2^T + v3 + bias                   [T,I]
  A_img  = softmax(v, axis=I)
  A_text = softmax(max(v, axis=I), axis=T)
  text_re = A_text^T @ text                    [1,D]
  img_re  = A_img @ img                        [T,D]
  G = concat([text, img_re, text*img_re, text*text_re], -1)   [T,4D]

Sharding: data-parallel over batch B=32 across 8 cores (4 batches/core),
weights replicated.

The kernel is HBM-bandwidth bound: 44 MB/core of traffic at ~330-360 GB/s
per core -> ~125-132 us floor. The schedule is organized so the DMA engines
never idle:
  - stores are split into three streams that become ready at different
    phases of a batch: G0 = text copy (ready at load), G12 = [img_re,
    text*img_re] per 128-row tile (ready per t-half as the attention matrix
    streams out of PE), and G4 = text*text_re (ready only after the full
    batch's max/sum reductions).
  - queues: SP ring carries text loads + G0/G12 stores (its sequencer is
    otherwise idle, so DMA dispatch holds never stall compute issue on
    ACT); Pool SWDGE carries img cast-loads and the G4 tail stores
    (grouped 2 tiles/DMA) so the batch-end bcast wait never head-blocks
    the SP ring. A pure-DMA probe (dma_probe.py) showed this split
    streams the full 44MB at the HBM floor (~122us) vs ~140us all-SP.
    Data-dependent stores must NEVER go on the ACT HWDGE ring: the ACT
    sequencer stalls on their readiness waits and the exp activations
    starve (+36us measured). DRAM->DRAM G0 is also a trap (per-row 2KB
    descriptors, +86us).
  - input pools are triple-buffered so loads run ~2 batches ahead;
    loads(b+2) are emitted before tail_bulk(b) so G4's bcast-chain wait
    cannot delay their ring dispatch.
  - the batch loop is software-pipelined: batch b+1's input-side prep
    (casts, imgT/rt, textT transposes, v2) is emitted during batch b's
    section so it fills PE/ACT slack there; the latency-critical scalar
    chain closing batch b's reductions (Z, 1/Z, text_re scale, broadcast)
    is emitted BEFORE that prep so it is never starved by it.

Device algorithm (transposed [I,T] layout so A_img never needs a
transpose):
  R[d,i]   = W3[d]*imgT[d,i] + W1[d]            (folds v1 into the matmul)
  vT[i,t]  = sum_d R[d,i]*textT[d,t]            (PE, bf16)
  expT     = exp(vT + (v2[i]+bias))             (ACT, bias is per-partition)
  s[t]     = sum_i expT  (PE matmul w/ ones);  img_re = expT^T @ img (PE)
  m'[t]    = max_i expT  (elementwise max over i-slabs + PE transpose +
             free-dim reduce);  A_text = m'/sum(m')  (exp is monotone)
  text_re  = (1/Z) * sum_t m'[t]*text[t,:]      (PE, rank-1 style)
"""

import numpy as np
import ml_dtypes

import concourse.bass as bass
import concourse.mybir as mybir
from concourse import bacc
from concourse.tile import TileContext
from concourse.bass_utils import run_bass_kernel_spmd

B, T, I, D = 32, 1024, 512, 512
N_CORES = 8
BPC = B // N_CORES  # batches per core

F32 = mybir.dt.float32
BF16 = mybir.dt.bfloat16

# build-time tuning knobs (read by _build_bass); _cache key includes them
OPTIONS = {
    "in_bufs": 3,  # text/img input pool slots (prefetch depth)
    "wk_bufs": 2,
    "sm_bufs": 3,  # per-batch working tiles
    "g12_bufs": 8,
    "g4_bufs": 4,
    "g0_q": "sp",
    "text_q": "sp",  # queue for the text load DMA
    "g0_defer": 99,  # batches >= this emit their G0 store at the batch tail  # queue for the G[:,0:D]=text store
    "g0_d2d": False,  # G0 as HBM->HBM DMA straight from text_in (no SBUF dep)
    "g12_q": "sp",
    "g12_alt": False,  # alternate G12 tile stores between sp and act rings
    "g12_pair": False,  # store two adjacent t-tiles per DMA (1MB transfers)
    "g12_split": False,
    "loads_early": True,  # emit loads(b+2) before tail_bulk(b) so G4 waits never head-block them
    "interleave": False,  # inject L(b+1)/G0(b+1) between G12 stores of main(b)
    "il_text_at": 1,  # G12 tile index after which the text load is injected
    "il_g0_at": 4,  # G12 tile index after which the G0 store is injected
    "g2_from_sbuf": False,  # G2 = gb[0:D]*text (frees ps_ir after G1 alone)  # queue for per-tile G[:,D:3D] stores
    "g1_eng": "act",  # engine for the G1 = ps_ir*rs PSUM->SBUF copy: act | dve | pool
    "loads_after_tail": False,  # emit loads(b+2) after tail_bulk(b): img Q7-gen no longer delays G4 triggers on the pool ring
    "g4_q": "pool",  # queue for per-tile G[:,3D:4D] stores
    "g4_alt": False,  # alternate G4 group stores between act and sp rings
    "g4_mul": "split",
    "last_split": False,
    "vt_merged": False,
    "ht": 2,  # t-dim pipeline slices (2 halves | 4 quarters)
    "mt_early": False,
    "g4_group": 2,  # tiles per G4 store DMA (1|2|4|8)  # engine(s) for text*text_re: dve | pool | split
    "ttcopy": "dve",
    "cast_eng": "dve",  # text f32->bf16 cast: dve | pool  # textT PSUM->SBUF copy engine: act | dve
    "hi_pri_loads": False,
    "img_hwdge": False,  # load img f32 on ACT HWDGE + DVE cast instead of SWDGE cast-DMA
    "ps_vt_bufs": 2,
    "ps_ir_bufs": 2,
    "ps_tr_bufs": 2,
    "ps_sm_bufs": 2,
}

_AF = mybir.ActivationFunctionType
_OP = mybir.AluOpType


def _build_bass(repeats=1):
    nc = bacc.Bacc()

    text_in = nc.dram_tensor("text_in", [BPC, T, D], F32, kind="ExternalInput")
    img_in = nc.dram_tensor("img_in", [BPC, I, D], F32, kind="ExternalInput")
    # host-folded weight constants, packed so each loads with ONE dma
    # consts_f cols: 0:4 w3c | 4:8 w1c | 8:9 bias | 9:137 ident_f | 137:138 ones_f
    consts_f = nc.dram_tensor("consts_f", [128, 138], F32, kind="ExternalInput")
    # consts_b cols: 0:4 w2 | 4:132 ident_b | 132:133 ones_b
    consts_b = nc.dram_tensor("consts_b", [128, 133], BF16, kind="ExternalInput")

    g_out = nc.dram_tensor("g_out", [BPC, T, 4 * D], F32, kind="ExternalOutput")

    NT = T // 128  # 8 t-tiles
    NI = I // 128  # 4 i-tiles
    NDC = D // 128  # 4 d-chunks
    HT = OPTIONS["ht"]  # t slices (pipeline granularity)
    TH = NT // HT  # tiles per slice
    HW = T // HT  # columns per slice

    def qeng(q):
        return {"sp": nc.sync, "act": nc.scalar, "pool": nc.gpsimd}[q]

    with TileContext(nc) as tc:
        with (
            tc.tile_pool(name="consts", bufs=1) as cpool,
            tc.tile_pool(name="inp", bufs=OPTIONS["in_bufs"]) as ipool,
            tc.tile_pool(name="wk", bufs=OPTIONS["wk_bufs"]) as wpool,
            tc.tile_pool(name="g12", bufs=OPTIONS["g12_bufs"]) as gpool12,
            tc.tile_pool(name="g4", bufs=OPTIONS["g4_bufs"]) as gpool4,
            tc.tile_pool(name="sm", bufs=OPTIONS["sm_bufs"]) as spool,
            tc.tile_pool(
                name="ps_vt", bufs=OPTIONS["ps_vt_bufs"], space="PSUM"
            ) as ps_vtp,
            tc.tile_pool(
                name="ps_ir", bufs=OPTIONS["ps_ir_bufs"], space="PSUM"
            ) as ps_irp,
            tc.tile_pool(
                name="ps_tr", bufs=OPTIONS["ps_tr_bufs"], space="PSUM"
            ) as ps_tr,
            tc.tile_pool(
                name="ps_sm", bufs=OPTIONS["ps_sm_bufs"], space="PSUM"
            ) as ps_sm,
        ):
            c_f = cpool.tile([128, 138], F32)
            c_b = cpool.tile([128, 133], BF16)
            with tc.high_priority():
                nc.sync.dma_start(c_f, consts_f[:, :])
                nc.sync.dma_start(c_b, consts_b[:, :])
            c_w3 = c_f[:, 0:4]
            c_w1 = c_f[:, 4:8]
            c_bias = c_f[:, 8:9]
            c_onesf = c_f[:, 137:138]
            c_w2 = c_b[:, 0:4]
            c_idb = c_b[:, 4:132]
            c_onesb = c_b[:, 132:133]

            def emit_text_load(b, st):
                qeng(OPTIONS["text_q"]).dma_start(
                    st["text_sb"], text_in[b].rearrange("(n p) d -> p n d", p=128)
                )

            def emit_g0(b, st):
                if OPTIONS["g0_d2d"]:
                    qeng(OPTIONS["g0_q"]).dma_start(g_out[b, :, 0:D], text_in[b])
                else:
                    gv = g_out[b].rearrange("(n p) g -> p n g", p=128)
                    qeng(OPTIONS["g0_q"]).dma_start(gv[:, :, 0:D], st["text_sb"])

            def emit_loads(b, defer_text=False):
                """Input DMAs + the G0 (=text) store; all ready-at-load."""
                st = {}
                st["text_sb"] = ipool.tile([128, NT, D], F32, tag="text_sb", name="text_sb")
                st["img_bf"] = ipool.tile([128, NI, D], BF16, tag="img_bf", name="img_bf")
                lp = tc.high_priority() if OPTIONS["hi_pri_loads"] else None
                if lp is not None:
                    lp.__enter__()
                if not defer_text:
                    emit_text_load(b, st)
                if OPTIONS["img_hwdge"]:
                    img_f32 = wpool.tile(
                        [128, NI, D], F32, tag="img_f32", name="img_f32"
                    )
                    nc.scalar.dma_start(
                        img_f32, img_in[b].rearrange("(m p) d -> p m d", p=128)
                    )
                    nc.vector.tensor_copy(st["img_bf"], img_f32)
                else:
                    nc.gpsimd.dma_start(
                        st["img_bf"], img_in[b].rearrange("(m p) d -> p m d", p=128)
                    )
                if lp is not None:
                    lp.__exit__(None, None, None)
                if b < OPTIONS["g0_defer"] and not defer_text:
                    emit_g0(b, st)
                return st

            def emit_prep(b, st):
                """Input-side prep for batch b, emitted one section early so it
                overlaps the previous batch's main phase."""
                text_sb = st["text_sb"]
                img_bf = st["img_bf"]
                text_bf = wpool.tile([128, NT, D], BF16, tag="text_bf")
                (nc.vector if OPTIONS["cast_eng"] == "dve" else nc.gpsimd).tensor_copy(
                    text_bf, text_sb
                )
                st["text_bf"] = text_bf

                rt_bf = wpool.tile([128, NDC, I], BF16, tag="rt_bf")
                imgT_bf = wpool.tile([128, NDC, I], BF16, tag="imgT_bf")
                for c in range(NDC):
                    ps_it = ps_tr.tile([128, I], BF16, tag="tr", name="ps_it")
                    for m in range(NI):
                        nc.tensor.transpose(
                            ps_it[:, m * 128 : (m + 1) * 128],
                            img_bf[:, m, c * 128 : (c + 1) * 128],
                            c_idb,
                        )
                    nc.vector.tensor_scalar(
                        rt_bf[:, c, :],
                        ps_it,
                        c_w3[:, c : c + 1],
                        c_w1[:, c : c + 1],
                        _OP.mult,
                        _OP.add,
                    )
                    nc.scalar.activation(imgT_bf[:, c, :], ps_it, _AF.Copy)
                st["rt_bf"] = rt_bf

                ps_v2 = ps_sm.tile([128, NI], F32, tag="ps", name="ps_v2")
                for m in range(NI):
                    for c in range(NDC):
                        nc.tensor.matmul(
                            ps_v2[:, m : m + 1],
                            imgT_bf[:, c, m * 128 : (m + 1) * 128],
                            c_w2[:, c : c + 1],
                            start=(c == 0),
                            stop=(c == NDC - 1),
                        )
                v2b = spool.tile([128, NI], F32, tag="v2b")
                nc.scalar.activation(v2b, ps_v2, _AF.Identity, bias=c_bias, scale=1.0)
                st["v2b"] = v2b

                textT_bf = wpool.tile([128, NDC, T], BF16, tag="textT_bf")
                for h in range(HT):
                    t0 = h * HW
                    for c in range(NDC):
                        ps_tt = ps_tr.tile([128, HW], BF16, tag="tr", name="ps_tt")
                        for k in range(TH):
                            n = h * TH + k
                            nc.tensor.transpose(
                                ps_tt[:, k * 128 : (k + 1) * 128],
                                text_bf[:, n, c * 128 : (c + 1) * 128],
                                c_idb,
                            )
                        if OPTIONS["ttcopy"] == "act":
                            nc.scalar.activation(
                                textT_bf[:, c, t0 : t0 + HW], ps_tt, _AF.Copy
                            )
                        else:
                            nc.vector.tensor_copy(textT_bf[:, c, t0 : t0 + HW], ps_tt)
                st["textT_bf"] = textT_bf

            def emit_main(b, st, fillers=None):
                text_sb = st["text_sb"]
                img_bf = st["img_bf"]
                textT_bf = st["textT_bf"]
                rt_bf = st["rt_bf"]
                v2b = st["v2b"]
                expT_bf = wpool.tile([128, NI, T], BF16, tag="expT_bf")
                m8 = spool.tile([128, T], BF16, tag="m8")
                mprime = spool.tile([128, NT], BF16, tag="mprime")
                st["mprime"] = mprime
                ps_trr = ps_sm.tile([1, 512], F32, tag="ps", name="ps_trr")
                st["ps_trr"] = ps_trr
                if OPTIONS["vt_merged"]:
                    # both t-halves per m-slab: consecutive matmuls share the
                    # same stationary rt chunk, halving the Ldweights count
                    for m in range(NI):
                        ps_vt0 = ps_vtp.tile([128, 512], F32, tag="vt", name="ps_vt0")
                        ps_vt1 = ps_vtp.tile([128, 512], F32, tag="vt", name="ps_vt1")
                        for c in range(NDC):
                            nc.tensor.matmul(
                                ps_vt0,
                                rt_bf[:, c, m * 128 : (m + 1) * 128],
                                textT_bf[:, c, 0:512],
                                start=(c == 0),
                                stop=(c == NDC - 1),
                                skip_group_check=True,
                            )
                            nc.tensor.matmul(
                                ps_vt1,
                                rt_bf[:, c, m * 128 : (m + 1) * 128],
                                textT_bf[:, c, 512:1024],
                                start=(c == 0),
                                stop=(c == NDC - 1),
                                skip_group_check=True,
                            )
                        nc.scalar.activation(
                            expT_bf[:, m, 0:512],
                            ps_vt0,
                            _AF.Exp,
                            bias=v2b[:, m : m + 1],
                            scale=1.0,
                        )
                        nc.scalar.activation(
                            expT_bf[:, m, 512:1024],
                            ps_vt1,
                            _AF.Exp,
                            bias=v2b[:, m : m + 1],
                            scale=1.0,
                        )
                for h in range(HT):
                    t0 = h * HW
                    if not OPTIONS["vt_merged"]:
                        for m in range(NI):
                            ps_vt = ps_vtp.tile([128, HW], F32, tag="vt", name="ps_vt")
                            for c in range(NDC):
                                nc.tensor.matmul(
                                    ps_vt,
                                    rt_bf[:, c, m * 128 : (m + 1) * 128],
                                    textT_bf[:, c, t0 : t0 + HW],
                                    start=(c == 0),
                                    stop=(c == NDC - 1),
                                )
                            nc.scalar.activation(
                                expT_bf[:, m, t0 : t0 + HW],
                                ps_vt,
                                _AF.Exp,
                                bias=v2b[:, m : m + 1],
                                scale=1.0,
                            )
                    mxa = spool.tile([128, HW], BF16, tag="mxa")
                    mxb = spool.tile([128, HW], BF16, tag="mxb")
                    nc.vector.tensor_max(
                        mxa, expT_bf[:, 0, t0 : t0 + HW], expT_bf[:, 1, t0 : t0 + HW]
                    )
                    nc.vector.tensor_max(
                        mxb, expT_bf[:, 2, t0 : t0 + HW], expT_bf[:, 3, t0 : t0 + HW]
                    )
                    nc.vector.tensor_max(m8[:, t0 : t0 + HW], mxa, mxb)

                    for k in range(TH):
                        n = h * TH + k
                        # ir and s interleaved per m-slab: consecutive
                        # matmuls share the same stationary lhsT, so the
                        # hardware skips the second Ldweights
                        ps_ir = ps_irp.tile([128, D], F32, tag="ir", name="ps_ir")
                        ps_s = ps_sm.tile([128, 1], F32, tag="ps", name="ps_s")
                        for m in range(NI):
                            nc.tensor.matmul(
                                ps_ir,
                                expT_bf[:, m, n * 128 : (n + 1) * 128],
                                img_bf[:, m, :],
                                start=(m == 0),
                                stop=(m == NI - 1),
                                skip_group_check=True,
                            )
                            nc.tensor.matmul(
                                ps_s,
                                expT_bf[:, m, n * 128 : (n + 1) * 128],
                                c_onesb,
                                start=(m == 0),
                                stop=(m == NI - 1),
                                skip_group_check=True,
                            )
                        rs = spool.tile([128, 1], F32, tag="rs")
                        nc.vector.reciprocal(rs, ps_s)

                        if OPTIONS["mt_early"]:
                            ps_mt = ps_sm.tile(
                                [128, 128], BF16, tag="ps", name="ps_mt"
                            )
                            nc.tensor.transpose(
                                ps_mt, m8[:, n * 128 : (n + 1) * 128], c_idb
                            )
                            nc.vector.reduce_max(
                                mprime[:, n : n + 1], ps_mt, axis=mybir.AxisListType.X
                            )
                            nc.tensor.matmul(
                                ps_trr,
                                mprime[:, n : n + 1],
                                st["text_bf"][:, n, :],
                                start=(n == 0),
                                stop=(n == NT - 1),
                                skip_group_check=True,
                            )

                        if OPTIONS["g12_pair"]:
                            if n % 2 == 0:
                                gbp = gpool12.tile([128, 2, 2 * D], F32, tag="g12")
                                st["gbp"] = gbp
                            else:
                                gbp = st["gbp"]
                            gb = gbp[:, n % 2, :]
                        else:
                            gb = gpool12.tile([128, 2 * D], F32, tag="g12")
                        if OPTIONS["g1_eng"] == "act":
                            nc.scalar.activation(
                                gb[:, 0:D], ps_ir, _AF.Copy, scale=rs
                            )
                        else:
                            g1e = (
                                nc.vector
                                if OPTIONS["g1_eng"] == "dve"
                                else nc.gpsimd
                            )
                            g1e.tensor_scalar(
                                gb[:, 0:D], ps_ir, rs, None, _OP.mult
                            )
                        g2eng = (
                            nc.gpsimd
                            if (b == BPC - 1 and n % 2 == 1 and OPTIONS["last_split"])
                            else nc.vector
                        )
                        if OPTIONS["g2_from_sbuf"]:
                            # text * img_re == text * G1: reading the already
                            # normalized SBUF copy releases the ps_ir bank
                            # after the ACT copy alone, so the next-but-one
                            # tile's matmuls start earlier
                            g2eng.tensor_mul(
                                gb[:, D : 2 * D], gb[:, 0:D], text_sb[:, n, :]
                            )
                        else:
                            g2eng.scalar_tensor_tensor(
                                gb[:, D : 2 * D],
                                ps_ir,
                                rs,
                                text_sb[:, n, :],
                                _OP.mult,
                                _OP.mult,
                            )
                        if OPTIONS["g12_alt"]:
                            g12q = "sp" if n % 2 == 0 else "act"
                        else:
                            g12q = OPTIONS["g12_q"]
                        if OPTIONS["g12_pair"]:
                            if n % 2 == 1:
                                gvp = g_out[b].rearrange("(q p) g -> p q g", p=128)
                                if OPTIONS["g12_alt"]:
                                    g12q = "sp" if (n // 2) % 2 == 0 else "act"
                                qeng(g12q).dma_start(
                                    gvp[:, n - 1 : n + 1, D : 3 * D], st["gbp"]
                                )
                        elif OPTIONS["g12_split"]:
                            qeng(g12q).dma_start(
                                g_out[b, n * 128 : (n + 1) * 128, D : 2 * D],
                                gb[:, 0:D],
                            )
                            qeng(g12q).dma_start(
                                g_out[b, n * 128 : (n + 1) * 128, 2 * D : 3 * D],
                                gb[:, D : 2 * D],
                            )
                        else:
                            qeng(g12q).dma_start(
                                g_out[b, n * 128 : (n + 1) * 128, D : 3 * D], gb
                            )

                        if fillers and n in fillers:
                            # inject ready-at-dispatch ring work (next batch's
                            # text load / G0 store) between G12 stores so the
                            # in-order SP ring has buffered SDMA work during
                            # the per-tile readiness waits
                            fillers[n]()

                        if not OPTIONS["mt_early"]:
                            ps_mt = ps_sm.tile(
                                [128, 128], BF16, tag="ps", name="ps_mt"
                            )
                            nc.tensor.transpose(
                                ps_mt, m8[:, n * 128 : (n + 1) * 128], c_idb
                            )
                            nc.vector.reduce_max(
                                mprime[:, n : n + 1], ps_mt, axis=mybir.AxisListType.X
                            )
                            # accumulate text_re per tile so the batch tail is
                            # only recip/scale/broadcast deep
                            nc.tensor.matmul(
                                ps_trr,
                                mprime[:, n : n + 1],
                                st["text_bf"][:, n, :],
                                start=(n == 0),
                                stop=(n == NT - 1),
                                skip_group_check=True,
                            )

            def emit_tail_fast(b, st):
                """Latency-critical scalar chain closing the batch's
                reductions; tiny engine time, so it outranks the next batch's
                prep in every engine's priority order."""
                mprime = st["mprime"]
                ps_trr = st["ps_trr"]
                mcol = spool.tile([128, 1], F32, tag="mcol")
                nc.vector.reduce_sum(mcol, mprime, axis=mybir.AxisListType.X)
                ps_z = ps_sm.tile([1, 1], F32, tag="ps", name="ps_z")
                nc.tensor.matmul(ps_z, mcol, c_onesf, start=True, stop=True)
                rz = spool.tile([1, 1], F32, tag="rz")
                nc.vector.reciprocal(rz, ps_z)
                trerow = spool.tile([1, 512], F32, tag="trerow")
                nc.scalar.activation(trerow, ps_trr, _AF.Copy, scale=rz)
                bcast = spool.tile([128, 512], F32, tag="bcast")
                nc.gpsimd.partition_broadcast(bcast, trerow)
                st["bcast"] = bcast

            def emit_tail_bulk(b, st):
                text_sb = st["text_sb"]
                bcast = st["bcast"]
                if b >= OPTIONS["g0_defer"]:
                    gv = g_out[b].rearrange("(n p) g -> p n g", p=128)
                    qeng(OPTIONS["g0_q"]).dma_start(gv[:, :, 0:D], text_sb)
                GG = OPTIONS["g4_group"]
                gv4 = g_out[b].rearrange("(q p) g -> p q g", p=128)
                for gi, n0 in enumerate(range(0, NT, GG)):
                    gb4 = gpool4.tile([128, GG, D], F32, tag="g4")
                    for j in range(GG):
                        n = n0 + j
                        mode = OPTIONS["g4_mul"]
                        if b == BPC - 1 and OPTIONS["last_split"]:
                            mode = "split"
                        if mode == "dve":
                            eng = nc.vector
                        elif mode == "pool":
                            eng = nc.gpsimd
                        else:
                            eng = nc.vector if n % 2 == 0 else nc.gpsimd
                        eng.tensor_mul(gb4[:, j, :], text_sb[:, n, :], bcast)
                    if OPTIONS["g4_alt"]:
                        g4q = "act" if gi % 2 == 0 else "sp"
                    else:
                        g4q = OPTIONS["g4_q"]
                    qeng(g4q).dma_start(gv4[:, n0 : n0 + GG, 3 * D : 4 * D], gb4)

            import contextlib

            loop_ctx = (
                tc.For_i(0, repeats, 1) if repeats > 1 else contextlib.nullcontext()
            )
            with loop_ctx:
                # software-pipelined: prep(b+1) is emitted during section b so
                # the next batch's transposes/casts overlap this batch's main
                # attention phase instead of serializing at the boundary.
                states = {0: emit_loads(0)}
                emit_prep(0, states[0])
                if OPTIONS["interleave"]:
                    # distance-1 prefetch with the text load / G0 store ring
                    # entries injected BETWEEN G12 stores of the current
                    # batch's main phase (in-order ring gap filling)
                    for b in range(BPC):
                        fillers = None
                        if b + 1 < BPC:
                            nst = states[b + 1] = emit_loads(
                                b + 1, defer_text=True
                            )
                            fillers = {
                                OPTIONS["il_text_at"]: (
                                    lambda b=b + 1, s=nst: emit_text_load(b, s)
                                ),
                                OPTIONS["il_g0_at"]: (
                                    lambda b=b + 1, s=nst: emit_g0(b, s)
                                ),
                            }
                        emit_main(b, states[b], fillers)
                        emit_tail_fast(b, states[b])
                        if b + 1 < BPC:
                            emit_prep(b + 1, states[b + 1])
                        emit_tail_bulk(b, states[b])
                        del states[b]
                elif OPTIONS["loads_early"]:
                    # loads(b+2) land on the rings BEFORE tail_bulk(b)'s G4
                    # stores, so the bcast-chain wait can't head-block them
                    if BPC > 1:
                        states[1] = emit_loads(1)
                    for b in range(BPC):
                        emit_main(b, states[b])
                        emit_tail_fast(b, states[b])
                        if not OPTIONS["loads_after_tail"] and b + 2 < BPC:
                            states[b + 2] = emit_loads(b + 2)
                        if b + 1 < BPC:
                            emit_prep(b + 1, states[b + 1])
                        emit_tail_bulk(b, states[b])
                        if OPTIONS["loads_after_tail"] and b + 2 < BPC:
                            states[b + 2] = emit_loads(b + 2)
                        del states[b]
                else:
                    for b in range(BPC):
                        if b + 1 < BPC:
                            states[b + 1] = emit_loads(b + 1)
                        emit_main(b, states[b])
                        emit_tail_fast(b, states[b])
                        if b + 1 < BPC:
                            emit_prep(b + 1, states[b + 1])
                        emit_tail_bulk(b, states[b])
                        del states[b]

    nc.compile()
    return nc


_cache = {}


def _get_nc(repeats=1):
    key = f"nc{repeats}-" + "-".join(f"{k}={v}" for k, v in sorted(OPTIONS.items()))
    if key not in _cache:
        _cache[key] = _build_bass(repeats)
    return _cache[key]


def _host_consts(W1, W2, W3, bias):
    w3c = W3[:, 0].reshape(4, 128).T.astype(np.float32)
    w1c = W1[:, 0].reshape(4, 128).T.astype(np.float32)
    w2c = W2[:, 0].reshape(4, 128).T.astype(np.float32)
    bias_col = np.full((128, 1), np.float32(bias[0]), dtype=np.float32)
    ident = np.eye(128, dtype=np.float32)
    ones = np.ones((128, 1), dtype=np.float32)
    consts_f = np.ascontiguousarray(
        np.concatenate([w3c, w1c, bias_col, ident, ones], axis=1, dtype=np.float32)
    )
    consts_b = np.ascontiguousarray(
        np.concatenate([w2c, ident, ones], axis=1).astype(ml_dtypes.bfloat16)
    )
    return dict(consts_f=consts_f, consts_b=consts_b)


def _run(inputs, trace=False, trace_kwargs=None):
    text = np.ascontiguousarray(np.asarray(inputs["text"], dtype=np.float32))
    img = np.ascontiguousarray(np.asarray(inputs["img"], dtype=np.float32))
    consts = _host_consts(
        np.asarray(inputs["W1"], dtype=np.float32),
        np.asarray(inputs["W2"], dtype=np.float32),
        np.asarray(inputs["W3"], dtype=np.float32),
        np.asarray(inputs["bias"], dtype=np.float32),
    )
    nc = _get_nc()
    in_maps = []
    for core in range(N_CORES):
        sl = slice(core * BPC, (core + 1) * BPC)
        in_maps.append(
            dict(
                text_in=np.ascontiguousarray(text[sl]),
                img_in=np.ascontiguousarray(img[sl]),
                **consts,
            )
        )
    kwargs = {}
    if trace:
        kwargs["trace"] = True
        if trace_kwargs:
            kwargs["trace_kwargs"] = trace_kwargs
    # The axon terminal is occasionally left in an "accelerator device
    # unrecoverable" state by a previous process; a backend reset + retry
    # reconnects to a healthy worker.
    last_exc = None
    for attempt in range(3):
        try:
            res = run_bass_kernel_spmd(
                nc, in_maps, core_ids=list(range(N_CORES)), **kwargs
            )
            break
        except Exception as e:  # noqa: BLE001
            last_exc = e
            if "UNRECOVERABLE" not in str(e) and "UNAVAILABLE" not in str(e):
                raise
            try:
                import jax
                import time as _time

                jax.clear_caches()
                jax._src.api.clear_backends()
                _time.sleep(5.0 * (attempt + 1))
            except Exception:
                pass
    else:
        raise last_exc
    out = np.concatenate([r["g_out"] for r in res.results], axis=0)
    return out, res


def kernel(**inputs) -> np.ndarray:
    out, _ = _run(inputs, trace=False)
    return out

